# revision 28
# baseline (speedup 1.0000x reference)
"""Multi-head attention (B=2, S=2048, D=1024, H=16, HD=64) on 8 trn2 cores.

Sharding: core c = (b, g) with b = c // 4 (batch), g = c % 4 (group of 4
heads).  Each core computes attention for its 4 heads of its batch and a
partial output projection; the host sums the 4 partials per batch and adds
the bias.

v2 vs v1: all matmul operands in bf16 (2-byte SBUF fetch; fp32r's 4-byte
moving-operand fetch ran ~2 cyc/col under engine contention), head-pair
packed score tiles (one exp per k-chunk covers 2 heads), exact causal
clipping (no widened +mask pieces), reciprocal_approx_fast for the softmax
denominators (DVE reciprocal() is ~6.5ns/elem), and projection / output-
projection matmuls injected between attention chunks as PE filler so the
tensor engine never idles waiting on exp.

Device-side dataflow (transposed layout):
  xT [D, S] bf16 (host-pre-transposed)
  Q^T = wq_g^T @ xT   [256, S]  (+ RoPE via pair-swap matmul + DVE combine)
  K^T = wk_g^T @ xT   [256, S]  (+ RoPE)
  V    per head [S, 64 | 1(ones)]  (x-stationary matmul -> [s, hd] layout)
  p^T[k, q] = exp((K^T chunk)^T @ Q^T / 8)  for head pair (2ch, 2ch+1) in
              one [128, 2, 512] psum tile; additive -1e5 tri mask on the
              leading 128 columns of diagonal-crossing chunks
  att^T[d, q], denom[q] = [V | 1]-stationary AV matmul accumulated over k
  att^T *= 1/denom  (reciprocal_approx_fast + gpsimd broadcast + DVE mul)
  out_partial[q, :] = att^T.T @ wo_g  -> bf16 -> DRAM
"""

import sys

if "/opt/trn_rl_repo" not in sys.path:
    sys.path.insert(0, "/opt/trn_rl_repo")

import ml_dtypes
import numpy as np

import concourse.bass as bass
import concourse.mybir as mybir
import concourse.tile as tile
from concourse import bacc
from concourse.bass_utils import run_bass_kernel_spmd

FP = mybir.dt.float32
BF = mybir.dt.bfloat16
BF_NP = ml_dtypes.bfloat16

B, S, D, H, HD = 2, 2048, 1024, 16, 64
NCORES = 8
GH = 4  # heads per core
GW = GH * HD  # 256: qkv columns / wo rows per core
ST = 512  # s-tile for projections
NST = S // ST
QT = 512  # q-tile in attention
KC = 128  # k-chunk in attention
DCH = D // 128  # 8 contraction chunks for projections
VW = HD + 2  # 66: per-head V stride (64 + ones col + pad)
NEG = -1.0e5  # additive causal mask (exp(0.125 * NEG) underflows to 0)


def _emit(nc, tc, xT, wq, wk, wv, wo, cosd, sind, rotm, trim, out):
    Exp = mybir.ActivationFunctionType.Exp
    PS = bass.MemorySpace.PSUM
    with (
        tc.tile_pool(name="const", bufs=1) as cpool,
        tc.tile_pool(name="wts", bufs=1) as wpool,
        tc.tile_pool(name="qkv", bufs=1) as qpool,
        tc.tile_pool(name="xin", bufs=2) as xpool,
        tc.tile_pool(name="wrk", bufs=2) as wrk,
        tc.tile_pool(name="psSC", bufs=2, space=PS) as psSC,
        tc.tile_pool(name="psAP", bufs=2, space=PS) as psAP,
        tc.tile_pool(name="psMS", bufs=2, space=PS) as psMS,
    ):
        # ---------------- constants / weights into SBUF ----------------
        # SP HWDGE ring (ordered): wq, xt0, wk, wv, xt1.. — gates matmuls.
        # ACT HWDGE ring (parallel): rot, cos, sin, tri, wo.
        xTr = xT.ap().rearrange("(c p) s -> p c s", p=128)

        def load_xt(st):
            t = xpool.tile([128, DCH, ST], BF, tag="xt")
            nc.sync.dma_start(t[:], xTr[:, :, st * ST : (st + 1) * ST])
            return t

        wq_sb = wpool.tile([128, DCH, GW], BF)
        nc.sync.dma_start(wq_sb[:], wq.ap())
        xts = [load_xt(0), None, None, None]
        wk_sb = wpool.tile([128, DCH, GW], BF)
        nc.sync.dma_start(wk_sb[:], wk.ap())
        wv_sb = wpool.tile([128, DCH, GW], BF)
        nc.sync.dma_start(wv_sb[:], wv.ap())

        rot_sb = cpool.tile([128, 128], BF)
        nc.scalar.dma_start(rot_sb[:], rotm.ap())
        cos_sb = cpool.tile([128, S], FP)
        nc.scalar.dma_start(cos_sb[:], cosd.ap())
        sin_sb = cpool.tile([128, S], FP)
        nc.scalar.dma_start(sin_sb[:], sind.ap())
        tri_sb = cpool.tile([128, 2, 128], FP)
        nc.scalar.dma_start(tri_sb[:], trim.ap())
        wo_sb = wpool.tile([128, 2, D], BF)
        nc.scalar.dma_start(wo_sb[:], wo.ap())

        # ---------------- persistent activations ----------------
        QTt = qpool.tile([128, 2, S], BF)  # roped Q^T (chunk ch = heads 2ch,2ch+1)
        KTt = qpool.tile([128, 2, S], BF)
        Vt = qpool.tile([128, S // KC, GH * VW], BF)  # [k%128, kc, head-major V]
        attT = qpool.tile([128, 2, S], BF)  # normalized att^T

        ones16 = cpool.tile([128, S // KC], FP)
        nc.vector.memset(ones16[:], 1.0)
        for h in range(GH):
            nc.vector.tensor_copy(Vt[:, :, h * VW + HD], ones16[:])

        # ---------------- projection / outproj units (PE filler) --------
        def qk_unit(st, wsb, dst, ch):
            def run():
                ssl = slice(st * ST, (st + 1) * ST)
                xt = xts[st]
                ps = psMS.tile([128, ST], FP, tag="ms")
                for dc in range(DCH):
                    nc.tensor.matmul(
                        ps[:],
                        wsb[:, dc, ch * 128 : (ch + 1) * 128],
                        xt[:, dc, :],
                        start=(dc == 0),
                        stop=(dc == DCH - 1),
                    )
                raw = wrk.tile([128, ST], BF, tag="raw")
                nc.scalar.copy(raw[:], ps[:])
                rps = psMS.tile([128, ST], FP, tag="ms")
                nc.tensor.matmul(rps[:], rot_sb[:], raw[:], start=True, stop=True)
                t1 = wrk.tile([128, ST], BF, tag="t1")
                nc.vector.tensor_mul(t1[:], ps[:], cos_sb[:, ssl])
                t2 = wrk.tile([128, ST], BF, tag="t2")
                nc.vector.tensor_mul(t2[:], rps[:], sin_sb[:, ssl])
                nc.vector.tensor_add(dst[:, ch, ssl], t1[:], t2[:])

            return run

        def v_unit(st, tb):
            def run():
                xt = xts[st]
                kc = st * (ST // 128) + tb
                ps = psMS.tile([128, GW], FP, tag="ms")
                for dc in range(DCH):
                    nc.tensor.matmul(
                        ps[:],
                        xt[:, dc, tb * 128 : (tb + 1) * 128],
                        wv_sb[:, dc, :],
                        start=(dc == 0),
                        stop=(dc == DCH - 1),
                    )
                dstap = Vt[:, kc].rearrange("p (h c) -> p h c", c=VW)[:, :, 0:HD]
                nc.vector.tensor_copy(dstap, ps[:].rearrange("p (h c) -> p h c", c=HD))

            return run

        def proj_units(st):
            us = []
            for wsb, dst in ((wq_sb, QTt), (wk_sb, KTt)):
                for ch in range(2):
                    us.append(qk_unit(st, wsb, dst, ch))
            for tb in range(ST // 128):
                us.append(v_unit(st, tb))
            return us

        def outproj_unit(qb):
            def run():
                for nt in range(2):
                    ops = psMS.tile([128, 512], FP, tag="ms")
                    for ch in range(2):
                        nc.tensor.matmul(
                            ops[:],
                            attT[:, ch, qb * 128 : (qb + 1) * 128],
                            wo_sb[:, ch, nt * 512 : (nt + 1) * 512],
                            start=(ch == 0),
                            stop=(ch == 1),
                        )
                    ob = wrk.tile([128, 512], BF, tag="ob", bufs=6)
                    nc.vector.tensor_copy(ob[:], ops[:])
                    for dh in range(2):  # half-width DMAs: 2x queue parallelism
                        c0 = nt * 512 + dh * 256
                        nc.gpsimd.dma_start(
                            out[qb * 128 : (qb + 1) * 128, c0 : c0 + 256],
                            ob[:, dh * 256 : (dh + 1) * 256],
                        )

            return run

        # ---------------- attention (one q-tile) ----------------
        def attn_qt(qt, units):
            qs = qt * QT
            nkc = (qs + QT) // KC
            n_iter = 2 * nkc
            n_done = 0
            it = 0
            for ch in range(2):
                aps = [
                    psAP.tile([HD + 1, QT], FP, tag="aps", name=f"aps{i}")
                    for i in range(2)
                ]
                for kc in range(nkc):
                    ks = kc * KC
                    off = max(0, ks - qs)
                    sps = psSC.tile([128, 2, QT], FP, tag="sc")
                    for hs in range(2):
                        r0 = hs * HD
                        nc.tensor.matmul(
                            sps[:, hs, off:QT],
                            KTt[r0 : r0 + HD, ch, ks : ks + KC],
                            QTt[r0 : r0 + HD, ch, qs + off : qs + QT],
                            start=True,
                            stop=True,
                        )
                    if ks >= qs:  # diagonal-crossing chunk: mask k > q
                        nc.vector.tensor_add(
                            sps[:, :, off : off + KC],
                            sps[:, :, off : off + KC],
                            tri_sb[:],
                        )
                    pt = wrk.tile([128, 2, QT], BF, tag="pt", bufs=4)
                    if off == 0:  # one contiguous 2D exp over both heads
                        nc.scalar.activation(
                            pt.rearrange("p a b -> p (a b)"),
                            sps.rearrange("p a b -> p (a b)"),
                            Exp,
                            scale=0.125,
                        )
                    else:
                        for hs in range(2):
                            nc.scalar.activation(
                                pt[:, hs, off:QT], sps[:, hs, off:QT], Exp, scale=0.125
                            )
                    it += 1
                    while units and n_done * n_iter <= it * len(units):
                        units.pop(0)()
                        n_done += 1
                    for hs in range(2):
                        h = 2 * ch + hs
                        nc.tensor.matmul(
                            aps[hs][:, off:QT],
                            Vt[:, kc, h * VW : h * VW + HD + 1],
                            pt[:, hs, off:QT],
                            start=(kc == 0),
                            stop=(kc == nkc - 1),
                        )
                den, rec = [], []
                for hs in range(2):
                    d = wrk.tile([1, QT], FP, tag="den", name=f"den{hs}")
                    nc.vector.tensor_copy(d[:], aps[hs][HD : HD + 1, :])
                    den.append(d)
                for hs in range(2):
                    r = wrk.tile([1, QT], FP, tag="rec", name=f"rec{hs}")
                    nc.vector.reciprocal_approx_fast(r[:], den[hs][:])
                    rec.append(r)
                bcs = []
                for hs in range(2):
                    b = wrk.tile([HD, QT], FP, tag="bcs", name=f"bcs{hs}")
                    nc.gpsimd.partition_broadcast(b[:], rec[hs][:])
                    bcs.append(b)
                for hs in range(2):
                    nc.vector.tensor_mul(
                        attT[hs * HD : hs * HD + HD, ch, qs : qs + QT],
                        aps[hs][0:HD, :],
                        bcs[hs][:],
                    )
            for u in units:  # leftovers (shouldn't happen)
                u()

        # ---------------- pipeline ----------------
        for u in proj_units(0):
            u()
        for qt in range(NST):
            if qt + 1 < NST:
                xts[qt + 1] = load_xt(qt + 1)
                units = proj_units(qt + 1)
            else:
                units = []
            if qt == 2:
                units.extend(outproj_unit(qb) for qb in range(0, 4))
            elif qt == 3:
                units.extend(outproj_unit(qb) for qb in range(4, 12))
            attn_qt(qt, units)
        for qb in range((NST - 1) * 4, NST * 4):
            outproj_unit(qb)()


_prog = None


def _build():
    global _prog
    if _prog is not None:
        return _prog
    nc = bacc.Bacc("TRN2", target_bir_lowering=False, debug=False)
    xT = nc.declare_dram_parameter("xT", [D, S], BF, isOutput=False)
    wq = nc.declare_dram_parameter("wq", [128, DCH, GW], BF, isOutput=False)
    wk = nc.declare_dram_parameter("wk", [128, DCH, GW], BF, isOutput=False)
    wv = nc.declare_dram_parameter("wv", [128, DCH, GW], BF, isOutput=False)
    wo = nc.declare_dram_parameter("wo", [128, 2, D], BF, isOutput=False)
    cosd = nc.declare_dram_parameter("cosd", [128, S], FP, isOutput=False)
    sind = nc.declare_dram_parameter("sind", [128, S], FP, isOutput=False)
    rotm = nc.declare_dram_parameter("rotm", [128, 128], BF, isOutput=False)
    trim = nc.declare_dram_parameter("trim", [128, 2, 128], FP, isOutput=False)
    out = nc.declare_dram_parameter("out", [S, D], BF, isOutput=True)
    with tile.TileContext(nc) as tc:
        _emit(nc, tc, xT, wq, wk, wv, wo, cosd, sind, rotm, trim, out)
    nc.compile()
    _prog = nc
    return nc


def _tables():
    inv = 1.0 / (10000.0 ** (np.arange(0, HD, 2)[: HD // 2].astype(np.float32) / HD))
    ang = np.outer(np.arange(S, dtype=np.float32), inv).astype(np.float32)  # [S, 32]
    cos64 = np.repeat(np.cos(ang).T, 2, axis=0).astype(np.float32)  # [64, S]
    sin64 = np.repeat(np.sin(ang).T, 2, axis=0).astype(np.float32)
    cos128 = np.tile(cos64, (2, 1)).astype(np.float32)
    sin128 = np.tile(sin64, (2, 1)).astype(np.float32)
    rotm = np.zeros((128, 128), np.float32)
    for f in range(64):
        rotm[2 * f + 1, 2 * f] = -1.0  # out[2f]   = -x[2f+1]
        rotm[2 * f, 2 * f + 1] = 1.0  # out[2f+1] = +x[2f]
    kk, qq = np.meshgrid(np.arange(128), np.arange(128), indexing="ij")
    tri = np.where(kk <= qq, 0.0, NEG).astype(np.float32)  # additive causal mask
    tri2 = np.ascontiguousarray(np.stack([tri, tri], axis=1))  # [128, 2, 128]
    return cos128, sin128, rotm.astype(BF_NP), tri2


def _pack_w(w):  # [D, GW] -> [128, DCH, GW], row dc*128+p -> [p, dc]
    return np.ascontiguousarray(
        np.asarray(w, np.float32).reshape(DCH, 128, GW).transpose(1, 0, 2)
    ).astype(BF_NP)


def _pack_wo(w):  # [GW, D] -> [128, 2, D]
    return np.ascontiguousarray(
        np.asarray(w, np.float32).reshape(2, 128, D).transpose(1, 0, 2)
    ).astype(BF_NP)


def make_in_maps(x, wq, wk, wv, wo_w):
    cos128, sin128, rotm, trim = _tables()
    in_maps = []
    for c in range(NCORES):
        b, g = divmod(c, GH)
        cs = slice(g * GW, (g + 1) * GW)
        in_maps.append(
            {
                "xT": np.ascontiguousarray(np.asarray(x[b], np.float32).T).astype(
                    BF_NP
                ),
                "wq": _pack_w(wq[:, cs]),
                "wk": _pack_w(wk[:, cs]),
                "wv": _pack_w(wv[:, cs]),
                "wo": _pack_wo(wo_w[cs, :]),
                "cosd": cos128,
                "sind": sin128,
                "rotm": rotm,
                "trim": trim,
            }
        )
    return in_maps


def kernel(x, wq, wk, wv, wo_w, wo_b):
    nc = _build()
    in_maps = make_in_maps(x, wq, wk, wv, wo_w)
    res = run_bass_kernel_spmd(nc, in_maps, list(range(NCORES))).results
    out = np.zeros((B, S, D), np.float32)
    for c in range(NCORES):
        out[c // GH] += np.asarray(res[c]["out"], dtype=np.float32)
    out += np.asarray(wo_b, np.float32)[None, None, :]
    return out
